# revision 16
# baseline (speedup 1.0000x reference)
"""Trainium2 Bass kernel for nn_MultiHeadSelfAttention (B=2, N=2048, C=1024, H=16).

Sharding: 8 cores = (batch b in {0,1}) x (head-group g in {0..3}); each core
computes 4 heads of one batch plus its partial output projection. The host
sums the 4 partial projections per batch and adds the bias constant
(v-bias and proj-bias folded together; k-bias is softmax-invariant and
dropped; q-bias applied on device).

Per-core pipeline (all layouts transposed so no on-chip transposes needed):
  qT,kT [256,2048] = W_{q,k} @ x^T
  v     [2048,256] = x @ W_v^T       (evicted to bf16, ones row appended)
  S'_h  [kv,q] = K_h q_h^T           (K=64 row-packed pairs)
  P'    = exp(S')                    (ACT, psum->sbuf, bf16 out)
  O^T_h [65,q] = [V_h | 1]^T P'_h    (row 64 = softmax denominator, free)
  O^T  /= D (broadcast D + recip), y_part = O^T.T @ W_p^T.

Steady-state schedule (KREPS>1): the rep loop is 2x-unrolled with ping-pong
q/k/v buffer sets; each half emits attention for parity p interleaved with
the qkv projection for parity 1-p, so the PE fills the gaps of the
ACT(exp)-paced attention window with the next rep's projections.
"""
import sys
import os

sys.path.insert(0, "/opt/trn_rl_repo")

import numpy as np
import ml_dtypes

import concourse.bass as bass
import concourse.mybir as mybir
from concourse import bacc
from concourse.tile import TileContext
from concourse.bass_utils import run_bass_kernel_spmd

F32R = mybir.dt.float32r
F32 = mybir.dt.float32
BF16 = mybir.dt.bfloat16
MM_BF16 = os.environ.get("KMMDT", "bf16") == "bf16"
MMDT = BF16 if MM_BF16 else F32R
Exp = mybir.ActivationFunctionType.Exp

B, N, C, H = 2, 2048, 1024, 16
HD = C // H          # 64
SCALE = 1.0 / np.sqrt(HD).astype(np.float32)

NQ = N // 512        # 4 q-chunks of 512
NK = N // 128        # 16 kv-chunks of 128
NJ = C // 128        # 8 contraction chunks for projections
NM = N // 128        # 16 token chunks


def build_nc():
    reps = int(os.environ.get("KREPS", "1"))
    assert reps == 1 or reps % 2 == 0, "KREPS must be 1 or even"
    nc = bacc.Bacc("TRN2", target_bir_lowering=False, debug=False, num_devices=8)

    xt_d = nc.dram_tensor("xt", [C, N], MMDT, kind="ExternalInput").ap()
    wqk_d = nc.dram_tensor("wqk", [128, NJ, 512], MMDT, kind="ExternalInput").ap()
    wv_d = nc.dram_tensor("wv", [128, NJ, 256], MMDT, kind="ExternalInput").ap()
    qb_d = nc.dram_tensor("qb", [128, 2], F32, kind="ExternalInput").ap()
    pw_d = nc.dram_tensor("pw", [128, 2, 1024], MMDT, kind="ExternalInput").ap()
    one_d = nc.dram_tensor("vones", [128, 64], BF16, kind="ExternalInput").ap()
    out_d = nc.dram_tensor("out", [N, C], F32, kind="ExternalOutput").ap()

    with TileContext(nc) as tc:
        with tc.tile_pool(name="const", bufs=1) as const, \
             tc.tile_pool(name="persist", bufs=1) as persist, \
             tc.tile_pool(name="pts", bufs=8) as pts, \
             tc.tile_pool(name="dsbp", bufs=3) as dsbp, \
             tc.tile_pool(name="yts", bufs=6) as yts, \
             tc.tile_pool(name="scr", bufs=1, space="DRAM") as scr, \
             tc.tile_pool(name="psS", bufs=2, space="PSUM") as psS, \
             tc.tile_pool(name="psO", bufs=2, space="PSUM") as psO, \
             tc.tile_pool(name="psC", bufs=2, space="PSUM") as psC:

            wqk_t = const.tile([128, NJ, 512], MMDT)
            wv_t = const.tile([128, NJ, 256], MMDT)
            qb_t = const.tile([128, 2], F32)
            pw_t = const.tile([128, 2, 1024], MMDT)
            xt_sb = const.tile([128, NJ, N], MMDT)   # x^T resident across reps

            npp = 2 if reps > 1 else 1   # ping-pong sets
            q_all = [const.tile([128, 2, N], MMDT, name=f"q_all{p}")
                     for p in range(npp)]
            k_all = [const.tile([128, 2, N], MMDT, name=f"k_all{p}")
                     for p in range(npp)]
            # v + ones row per head slot s (s = 2*hp + parity): [kv, i, s, 65]
            v_all = [const.tile([128, NK, 4, 65], BF16, name=f"v_all{p}")
                     for p in range(npp)]
            ou_all = persist.tile([128, 2, N], F32)   # unnormalized O^T
            db_all = persist.tile([128, 2, N], F32)   # broadcast 1/D
            on_all = persist.tile([128, 2, N], MMDT)  # normalized O^T

            # constants + ones columns (v evictions never touch col 64, so
            # this survives every rep)
            nc.scalar.dma_start(out=qb_t, in_=qb_d)
            nc.scalar.dma_start(out=pw_t, in_=pw_d)
            for j in range(NJ):
                nc.scalar.dma_start(out=wqk_t[:, j, :], in_=wqk_d[:, j, :])
                nc.scalar.dma_start(out=wv_t[:, j, :], in_=wv_d[:, j, :])
            for p in range(npp):
                nc.scalar.dma_start(
                    out=v_all[p][:, :, :, 64:65].rearrange("p a b c -> p (a b c)"),
                    in_=one_d)
            for j in range(NJ):
                nc.sync.dma_start(out=xt_sb[:, j, :],
                                  in_=xt_d[128 * j:128 * (j + 1), :])

            def emit_qkv_n(par, n):
                """Project q,k,v for 512-token chunk n into buffer set par.
                Eight single-bank accumulation groups pipelined through two
                [128,512] psC slots. k/v first: they gate the next half's
                first S chunks, while q(n) is only needed at window n."""
                nsl = slice(512 * n, 512 * (n + 1))
                xts = [xt_sb[:, j, nsl] for j in range(NJ)]
                # k: feature tiles m=0,1 (rows 256:512 of wqk)
                for m in range(2):
                    pk = psC.tile([128, 512], F32, tag="psC", name=f"pk{m}")
                    for j in range(NJ):
                        nc.tensor.matmul(pk,
                                         lhsT=wqk_t[:, j, 128 * (m + 2):128 * (m + 3)],
                                         rhs=xts[j], start=(j == 0),
                                         stop=(j == NJ - 1))
                    nc.vector.tensor_copy(out=k_all[par][:, m, nsl], in_=pk)
                # v: token tiles t=0..3, one 256-wide group per bank slot
                for t in range(4):
                    pv = psC.tile([128, 512], F32, tag="psC", name=f"pv{t}")
                    for j in range(NJ):
                        nc.tensor.matmul(
                            pv[:, 0:256],
                            lhsT=xts[j][:, 128 * t:128 * (t + 1)],
                            rhs=wv_t[:, j, :], start=(j == 0),
                            stop=(j == NJ - 1))
                    nc.vector.tensor_copy(
                        out=v_all[par][:, 4 * n + t, :, 0:64],
                        in_=pv[:, 0:256].rearrange("p (s f) -> p s f", s=4))
                # q: feature tiles m=0,1
                for m in range(2):
                    pq = psC.tile([128, 512], F32, tag="psC", name=f"pq{m}")
                    for j in range(NJ):
                        nc.tensor.matmul(pq,
                                         lhsT=wqk_t[:, j, 128 * m:128 * (m + 1)],
                                         rhs=xts[j], start=(j == 0),
                                         stop=(j == NJ - 1))
                    nc.vector.tensor_scalar_add(
                        out=q_all[par][:, m, nsl], in0=pq,
                        scalar1=qb_t[:, m:m + 1])

            def emit_attn_n(par, n):
                """Attention + normalize + y-projection for q-chunk n on
                buffer set par."""
                nsl = slice(512 * n, 512 * (n + 1))
                for hp in range(2):
                    oe_ps = psO.tile([128, 512], F32, tag="psO", name="oe_ps")
                    oo_ps = psO.tile([128, 512], F32, tag="psO", name="oo_ps")
                    oe = oe_ps[0:65, :]
                    oo = oo_ps[0:65, :]
                    for i in range(NK):
                        s2 = psS.tile([128, 1024], F32, tag="psS", name="s2")
                        isl = slice(128 * i, 128 * (i + 1))
                        nc.tensor.matmul(s2[:, 0:512],
                                         lhsT=k_all[par][0:64, hp, isl],
                                         rhs=q_all[par][0:64, hp, nsl],
                                         start=True, stop=True)
                        nc.tensor.matmul(s2[:, 512:1024],
                                         lhsT=k_all[par][64:128, hp, isl],
                                         rhs=q_all[par][64:128, hp, nsl],
                                         start=True, stop=True)
                        pt = pts.tile([128, 1024], BF16, tag="pt")
                        nc.scalar.activation(out=pt, in_=s2, func=Exp)
                        nc.tensor.matmul(oe, lhsT=v_all[par][:, i, 2 * hp, :],
                                         rhs=pt[:, 0:512], start=(i == 0),
                                         stop=(i == NK - 1))
                        nc.tensor.matmul(oo, lhsT=v_all[par][:, i, 2 * hp + 1, :],
                                         rhs=pt[:, 512:1024], start=(i == 0),
                                         stop=(i == NK - 1))
                    # evict unnormalized O^T first: releases the psO banks
                    nc.vector.tensor_copy(out=ou_all[0:64, hp, nsl],
                                          in_=oe[0:64, :])
                    nc.vector.tensor_copy(out=ou_all[64:128, hp, nsl],
                                          in_=oo[0:64, :])
                    # denominators (row 64): psum -> sbuf (recip'd) ->
                    # gpsimd partition-broadcast, then mul. The Q7 ucode
                    # requires src AND dst to start at partition 0, so: put
                    # both D rows at partition 0 of their own tiles,
                    # broadcast 1/D_odd to all 128 rows, then overwrite
                    # rows 0:64 with 1/D_even.
                    dse = dsbp.tile([1, 512], F32, tag="dse", name="dse")
                    dso = dsbp.tile([1, 512], F32, tag="dso", name="dso")
                    nc.vector.tensor_copy(out=dse, in_=oe[64:65, :])
                    nc.vector.tensor_copy(out=dso, in_=oo[64:65, :])
                    nc.vector.reciprocal_approx_fast(out=dse, in_=dse)
                    nc.vector.reciprocal_approx_fast(out=dso, in_=dso)
                    nc.gpsimd.partition_broadcast(
                        db_all[:, hp, nsl], dso, channels=128)
                    nc.gpsimd.partition_broadcast(
                        db_all[0:64, hp, nsl], dse, channels=64)
                    nc.vector.tensor_mul(
                        out=on_all[:, hp, nsl], in0=ou_all[:, hp, nsl],
                        in1=db_all[:, hp, nsl])
                # y-projection for the 4 token chunks of this n
                for mm_ in range(4):
                    m = 4 * n + mm_
                    for nn in range(2):
                        py = psO.tile([128, 512], F32, tag="psO", name="py")
                        for hp in range(2):
                            nc.tensor.matmul(
                                py, lhsT=on_all[:, hp, 128 * m:128 * (m + 1)],
                                rhs=pw_t[:, hp, 512 * nn:512 * (nn + 1)],
                                start=(hp == 0), stop=(hp == 1))
                        yt = yts.tile([128, 512], F32, tag="yt")
                        nc.vector.tensor_copy(out=yt, in_=py)
                        nc.sync.dma_start(
                            out=out_d[128 * m:128 * (m + 1),
                                      512 * nn:512 * (nn + 1)],
                            in_=yt)

            # prologue: fill buffer set 0
            for n in range(NQ):
                emit_qkv_n(0, n)

            if reps == 1:
                for n in range(NQ):
                    emit_attn_n(0, n)
            else:
                unroll = max((u for u in (8, 4, 2) if reps % u == 0),
                             default=2)
                with tc.For_i(0, reps // unroll, 1,
                              hint_engines=(mybir.EngineType.PE,
                                            mybir.EngineType.SP)):
                    for par in [0, 1] * (unroll // 2):
                        for n in range(NQ):
                            emit_attn_n(par, n)
                            emit_qkv_n(1 - par, n)

    nc.finalize()
    return nc


_NC = None


def _get_nc():
    global _NC
    if _NC is None:
        _NC = build_nc()
    return _NC


def make_in_maps(x, qkv_w, qkv_b, proj_w):
    """Host-side shard prep. Core c = 4*b + g handles batch b, heads 4g..4g+3."""
    x = np.asarray(x, np.float32)
    qkv_w = np.asarray(qkv_w, np.float32)
    qkv_b = np.asarray(qkv_b, np.float32)
    proj_w = np.asarray(proj_w, np.float32)
    in_maps = []
    vones = np.ones((128, 64), dtype=ml_dtypes.bfloat16)
    for c in range(8):
        b, g = divmod(c, 4)
        hs = g * 4 * HD  # 256-wide feature slice for this core's heads
        xt = np.ascontiguousarray(x[b].T)                       # [C, N]
        wq = qkv_w[hs:hs + 256, :] * SCALE                      # pre-scaled q
        wk = qkv_w[C + hs:C + hs + 256, :]
        wqkT = np.ascontiguousarray(np.concatenate([wq, wk], 0).T)   # [C, 512]
        wqk = np.ascontiguousarray(wqkT.reshape(NJ, 128, 512).transpose(1, 0, 2))
        wvT = np.ascontiguousarray(qkv_w[2 * C + hs:2 * C + hs + 256, :].T)
        wv = np.ascontiguousarray(wvT.reshape(NJ, 128, 256).transpose(1, 0, 2))
        qb = np.ascontiguousarray((qkv_b[hs:hs + 256] * SCALE).reshape(2, 128).T)
        pwT = np.ascontiguousarray(proj_w[:, hs:hs + 256].T)    # [256, C]
        pw = np.ascontiguousarray(pwT.reshape(2, 128, 1024).transpose(1, 0, 2))
        if MM_BF16:
            bf = ml_dtypes.bfloat16
            xt, wqk, wv, pw = (a.astype(bf) for a in (xt, wqk, wv, pw))
        in_maps.append({"xt": xt, "wqk": wqk, "wv": wv, "qb": qb, "pw": pw,
                        "vones": vones})
    return in_maps


def unshard(results, qkv_b, proj_w, proj_b):
    cvec = (np.asarray(qkv_b, np.float32)[2 * C:] @ np.asarray(proj_w, np.float32).T
            + np.asarray(proj_b, np.float32))
    y = np.empty((B, N, C), np.float32)
    for b in range(B):
        acc = results[4 * b]["out"].copy()
        for g in range(1, 4):
            acc += results[4 * b + g]["out"]
        y[b] = acc + cvec[None, :]
    return y


def kernel(x, qkv_w, qkv_b, proj_w, proj_b):
    nc = _get_nc()
    in_maps = make_in_maps(x, qkv_w, qkv_b, proj_w)
    res = run_bass_kernel_spmd(nc, in_maps, core_ids=list(range(8)))
    return unshard(res.results, qkv_b, proj_w, proj_b)


# revision 17
# speedup vs baseline: 1.0089x; 1.0089x over previous
"""Trainium2 Bass kernel for nn_MultiHeadSelfAttention (B=2, N=2048, C=1024, H=16).

Sharding: 8 cores = (batch b in {0,1}) x (head-group g in {0..3}); each core
computes 4 heads of one batch plus its partial output projection. The host
sums the 4 partial projections per batch and adds the bias constant
(v-bias and proj-bias folded together; k-bias is softmax-invariant and
dropped; q-bias applied on device).

Per-core pipeline (all layouts transposed so no on-chip transposes needed):
  qT,kT [256,2048] = W_{q,k} @ x^T
  v     [2048,256] = x @ W_v^T       (evicted to bf16, ones row appended)
  S'_h  [kv,q] = K_h q_h^T           (K=64 row-packed pairs)
  P'    = exp(S')                    (ACT, psum->sbuf, bf16 out)
  O^T_h [65,q] = [V_h | 1]^T P'_h    (row 64 = softmax denominator, free)
  O^T  /= D (broadcast D + recip), y_part = O^T.T @ W_p^T.

Steady-state schedule (KREPS>1): the rep loop is 2x-unrolled with ping-pong
q/k/v buffer sets; each half emits attention for parity p interleaved with
the qkv projection for parity 1-p, so the PE fills the gaps of the
ACT(exp)-paced attention window with the next rep's projections.
"""
import sys
import os

sys.path.insert(0, "/opt/trn_rl_repo")

import numpy as np
import ml_dtypes

import concourse.bass as bass
import concourse.mybir as mybir
from concourse import bacc
from concourse.tile import TileContext
from concourse.bass_utils import run_bass_kernel_spmd

F32R = mybir.dt.float32r
F32 = mybir.dt.float32
BF16 = mybir.dt.bfloat16
MM_BF16 = os.environ.get("KMMDT", "bf16") == "bf16"
MMDT = BF16 if MM_BF16 else F32R
Exp = mybir.ActivationFunctionType.Exp

B, N, C, H = 2, 2048, 1024, 16
HD = C // H          # 64
SCALE = 1.0 / np.sqrt(HD).astype(np.float32)

NQ = N // 512        # 4 q-chunks of 512
NK = N // 128        # 16 kv-chunks of 128
NJ = C // 128        # 8 contraction chunks for projections
NM = N // 128        # 16 token chunks


def build_nc():
    reps = int(os.environ.get("KREPS", "1"))
    assert reps == 1 or reps % 2 == 0, "KREPS must be 1 or even"
    nc = bacc.Bacc("TRN2", target_bir_lowering=False, debug=False, num_devices=8)

    xt_d = nc.dram_tensor("xt", [C, N], MMDT, kind="ExternalInput").ap()
    wqk_d = nc.dram_tensor("wqk", [128, NJ, 512], MMDT, kind="ExternalInput").ap()
    wv_d = nc.dram_tensor("wv", [128, NJ, 256], MMDT, kind="ExternalInput").ap()
    qb_d = nc.dram_tensor("qb", [128, 2], F32, kind="ExternalInput").ap()
    pw_d = nc.dram_tensor("pw", [128, 2, 1024], MMDT, kind="ExternalInput").ap()
    one_d = nc.dram_tensor("vones", [128, 64], BF16, kind="ExternalInput").ap()
    out_d = nc.dram_tensor("out", [N, C], F32, kind="ExternalOutput").ap()

    with TileContext(nc) as tc:
        with tc.tile_pool(name="const", bufs=1) as const, \
             tc.tile_pool(name="persist", bufs=1) as persist, \
             tc.tile_pool(name="pts", bufs=8) as pts, \
             tc.tile_pool(name="dsbp", bufs=3) as dsbp, \
             tc.tile_pool(name="yts", bufs=6) as yts, \
             tc.tile_pool(name="scr", bufs=1, space="DRAM") as scr, \
             tc.tile_pool(name="psS", bufs=2, space="PSUM") as psS, \
             tc.tile_pool(name="psO", bufs=2, space="PSUM") as psO, \
             tc.tile_pool(name="psC", bufs=2, space="PSUM") as psC:

            wqk_t = const.tile([128, NJ, 512], MMDT)
            wv_t = const.tile([128, NJ, 256], MMDT)
            qb_t = const.tile([128, 2], F32)
            pw_t = const.tile([128, 2, 1024], MMDT)
            xt_sb = const.tile([128, NJ, N], MMDT)   # x^T resident across reps

            npp = 2 if reps > 1 else 1   # ping-pong sets
            q_all = [const.tile([128, 2, N], MMDT, name=f"q_all{p}")
                     for p in range(npp)]
            k_all = [const.tile([128, 2, N], MMDT, name=f"k_all{p}")
                     for p in range(npp)]
            # v + ones row per head slot s (s = 2*hp + parity): [kv, i, s, 65]
            v_all = [const.tile([128, NK, 4, 65], BF16, name=f"v_all{p}")
                     for p in range(npp)]
            ou_all = persist.tile([128, 2, N], F32)   # unnormalized O^T
            db_all = persist.tile([128, 2, N], F32)   # broadcast 1/D
            on_all = persist.tile([128, 2, N], MMDT)  # normalized O^T

            # constants + ones columns (v evictions never touch col 64, so
            # this survives every rep)
            nc.scalar.dma_start(out=qb_t, in_=qb_d)
            nc.scalar.dma_start(out=pw_t, in_=pw_d)
            for j in range(NJ):
                nc.scalar.dma_start(out=wqk_t[:, j, :], in_=wqk_d[:, j, :])
                nc.scalar.dma_start(out=wv_t[:, j, :], in_=wv_d[:, j, :])
            for p in range(npp):
                nc.scalar.dma_start(
                    out=v_all[p][:, :, :, 64:65].rearrange("p a b c -> p (a b c)"),
                    in_=one_d)
            for j in range(NJ):
                nc.sync.dma_start(out=xt_sb[:, j, :],
                                  in_=xt_d[128 * j:128 * (j + 1), :])

            def emit_qkv_n(par, n):
                """Project q,k,v for 512-token chunk n into buffer set par.
                Eight single-bank accumulation groups pipelined through two
                [128,512] psC slots. k/v first: they gate the next half's
                first S chunks, while q(n) is only needed at window n."""
                nsl = slice(512 * n, 512 * (n + 1))
                xts = [xt_sb[:, j, nsl] for j in range(NJ)]
                # k: feature tiles m=0,1 (rows 256:512 of wqk)
                for m in range(2):
                    pk = psC.tile([128, 512], F32, tag="psC", name=f"pk{m}")
                    for j in range(NJ):
                        nc.tensor.matmul(pk,
                                         lhsT=wqk_t[:, j, 128 * (m + 2):128 * (m + 3)],
                                         rhs=xts[j], start=(j == 0),
                                         stop=(j == NJ - 1))
                    nc.vector.tensor_copy(out=k_all[par][:, m, nsl], in_=pk)
                # v: token tiles t=0..3, one 256-wide group per bank slot
                for t in range(4):
                    pv = psC.tile([128, 512], F32, tag="psC", name=f"pv{t}")
                    for j in range(NJ):
                        nc.tensor.matmul(
                            pv[:, 0:256],
                            lhsT=xts[j][:, 128 * t:128 * (t + 1)],
                            rhs=wv_t[:, j, :], start=(j == 0),
                            stop=(j == NJ - 1))
                    nc.vector.tensor_copy(
                        out=v_all[par][:, 4 * n + t, :, 0:64],
                        in_=pv[:, 0:256].rearrange("p (s f) -> p s f", s=4))
                # q: feature tiles m=0,1
                for m in range(2):
                    pq = psC.tile([128, 512], F32, tag="psC", name=f"pq{m}")
                    for j in range(NJ):
                        nc.tensor.matmul(pq,
                                         lhsT=wqk_t[:, j, 128 * m:128 * (m + 1)],
                                         rhs=xts[j], start=(j == 0),
                                         stop=(j == NJ - 1))
                    nc.vector.tensor_scalar_add(
                        out=q_all[par][:, m, nsl], in0=pq,
                        scalar1=qb_t[:, m:m + 1])

            def emit_attn_n(par, n):
                """Attention + normalize + y-projection for q-chunk n on
                buffer set par."""
                nsl = slice(512 * n, 512 * (n + 1))
                for hp in range(2):
                    oe_ps = psO.tile([128, 512], F32, tag="psO", name="oe_ps")
                    oo_ps = psO.tile([128, 512], F32, tag="psO", name="oo_ps")
                    oe = oe_ps[0:65, :]
                    oo = oo_ps[0:65, :]
                    for i in range(NK):
                        s2 = psS.tile([128, 1024], F32, tag="psS", name="s2")
                        isl = slice(128 * i, 128 * (i + 1))
                        nc.tensor.matmul(s2[:, 0:512],
                                         lhsT=k_all[par][0:64, hp, isl],
                                         rhs=q_all[par][0:64, hp, nsl],
                                         start=True, stop=True)
                        nc.tensor.matmul(s2[:, 512:1024],
                                         lhsT=k_all[par][64:128, hp, isl],
                                         rhs=q_all[par][64:128, hp, nsl],
                                         start=True, stop=True)
                        pt = pts.tile([128, 1024], BF16, tag="pt")
                        nc.scalar.activation(out=pt, in_=s2, func=Exp)
                        nc.tensor.matmul(oe, lhsT=v_all[par][:, i, 2 * hp, :],
                                         rhs=pt[:, 0:512], start=(i == 0),
                                         stop=(i == NK - 1))
                        nc.tensor.matmul(oo, lhsT=v_all[par][:, i, 2 * hp + 1, :],
                                         rhs=pt[:, 512:1024], start=(i == 0),
                                         stop=(i == NK - 1))
                    # evict unnormalized O^T first: releases the psO banks
                    nc.vector.tensor_copy(out=ou_all[0:64, hp, nsl],
                                          in_=oe[0:64, :])
                    nc.vector.tensor_copy(out=ou_all[64:128, hp, nsl],
                                          in_=oo[0:64, :])
                    # denominators (row 64): psum -> sbuf (recip'd) ->
                    # gpsimd partition-broadcast, then mul. The Q7 ucode
                    # requires src AND dst to start at partition 0, so: put
                    # both D rows at partition 0 of their own tiles,
                    # broadcast 1/D_odd to all 128 rows, then overwrite
                    # rows 0:64 with 1/D_even.
                    dse = dsbp.tile([1, 512], F32, tag="dse", name="dse")
                    dso = dsbp.tile([1, 512], F32, tag="dso", name="dso")
                    nc.vector.tensor_copy(out=dse, in_=oe[64:65, :])
                    nc.vector.tensor_copy(out=dso, in_=oo[64:65, :])
                    nc.vector.reciprocal_approx_fast(out=dse, in_=dse)
                    nc.vector.reciprocal_approx_fast(out=dso, in_=dso)
                    nc.gpsimd.partition_broadcast(
                        db_all[:, hp, nsl], dso, channels=128)
                    nc.gpsimd.partition_broadcast(
                        db_all[0:64, hp, nsl], dse, channels=64)
                    nc.vector.tensor_mul(
                        out=on_all[:, hp, nsl], in0=ou_all[:, hp, nsl],
                        in1=db_all[:, hp, nsl])
                # y-projection for the 4 token chunks of this n
                for mm_ in range(4):
                    m = 4 * n + mm_
                    for nn in range(2):
                        py = psO.tile([128, 512], F32, tag="psO", name="py")
                        for hp in range(2):
                            nc.tensor.matmul(
                                py, lhsT=on_all[:, hp, 128 * m:128 * (m + 1)],
                                rhs=pw_t[:, hp, 512 * nn:512 * (nn + 1)],
                                start=(hp == 0), stop=(hp == 1))
                        yt = yts.tile([128, 512], F32, tag="yt")
                        nc.vector.tensor_copy(out=yt, in_=py)
                        nc.sync.dma_start(
                            out=out_d[128 * m:128 * (m + 1),
                                      512 * nn:512 * (nn + 1)],
                            in_=yt)

            # prologue: fill buffer set 0
            for n in range(NQ):
                emit_qkv_n(0, n)

            if reps == 1:
                for n in range(NQ):
                    emit_attn_n(0, n)
            else:
                unroll = 4 if reps % 4 == 0 else 2
                with tc.For_i(0, reps // unroll, 1,
                              hint_engines=(mybir.EngineType.PE,
                                            mybir.EngineType.SP)):
                    for par in [0, 1] * (unroll // 2):
                        for n in range(NQ):
                            emit_attn_n(par, n)
                            emit_qkv_n(1 - par, n)

    nc.finalize()
    return nc


_NC = None


def _get_nc():
    global _NC
    if _NC is None:
        _NC = build_nc()
    return _NC


def make_in_maps(x, qkv_w, qkv_b, proj_w):
    """Host-side shard prep. Core c = 4*b + g handles batch b, heads 4g..4g+3."""
    x = np.asarray(x, np.float32)
    qkv_w = np.asarray(qkv_w, np.float32)
    qkv_b = np.asarray(qkv_b, np.float32)
    proj_w = np.asarray(proj_w, np.float32)
    in_maps = []
    vones = np.ones((128, 64), dtype=ml_dtypes.bfloat16)
    for c in range(8):
        b, g = divmod(c, 4)
        hs = g * 4 * HD  # 256-wide feature slice for this core's heads
        xt = np.ascontiguousarray(x[b].T)                       # [C, N]
        wq = qkv_w[hs:hs + 256, :] * SCALE                      # pre-scaled q
        wk = qkv_w[C + hs:C + hs + 256, :]
        wqkT = np.ascontiguousarray(np.concatenate([wq, wk], 0).T)   # [C, 512]
        wqk = np.ascontiguousarray(wqkT.reshape(NJ, 128, 512).transpose(1, 0, 2))
        wvT = np.ascontiguousarray(qkv_w[2 * C + hs:2 * C + hs + 256, :].T)
        wv = np.ascontiguousarray(wvT.reshape(NJ, 128, 256).transpose(1, 0, 2))
        qb = np.ascontiguousarray((qkv_b[hs:hs + 256] * SCALE).reshape(2, 128).T)
        pwT = np.ascontiguousarray(proj_w[:, hs:hs + 256].T)    # [256, C]
        pw = np.ascontiguousarray(pwT.reshape(2, 128, 1024).transpose(1, 0, 2))
        if MM_BF16:
            bf = ml_dtypes.bfloat16
            xt, wqk, wv, pw = (a.astype(bf) for a in (xt, wqk, wv, pw))
        in_maps.append({"xt": xt, "wqk": wqk, "wv": wv, "qb": qb, "pw": pw,
                        "vones": vones})
    return in_maps


def unshard(results, qkv_b, proj_w, proj_b):
    cvec = (np.asarray(qkv_b, np.float32)[2 * C:] @ np.asarray(proj_w, np.float32).T
            + np.asarray(proj_b, np.float32))
    y = np.empty((B, N, C), np.float32)
    for b in range(B):
        acc = results[4 * b]["out"].copy()
        for g in range(1, 4):
            acc += results[4 * b + g]["out"]
        y[b] = acc + cvec[None, :]
    return y


def kernel(x, qkv_w, qkv_b, proj_w, proj_b):
    nc = _get_nc()
    in_maps = make_in_maps(x, qkv_w, qkv_b, proj_w)
    res = run_bass_kernel_spmd(nc, in_maps, core_ids=list(range(8)))
    return unshard(res.results, qkv_b, proj_w, proj_b)
